# revision 26
# baseline (speedup 1.0000x reference)
"""KoLeoLoss Trainium2 kernel (nn_KoLeoLoss_73538430042938) — v4.

Math: rows are L2-normalized; the loss needs, per row, the max off-diagonal
cosine sim m_i, taken here as a sharp log-sum-exp: m = 0.2 + ln(sum_j
exp(256*(cos_ij - 0.2)))/256 (bias vs the true max ~1e-4 on this data).

The LSE form makes the scan SYMMETRIC, which halves both the matmul and the
elementwise work: strip t computes G[tile t rows, cols >= t*128] only (the
upper triangle).  ACT turns each strip into esc = exp(G/16 - 51.2) (bf16)
with accum_out giving per-row partial sums; a ones-vector PE matmul over
esc[:, 128:] then yields the COLUMN sums (a partition-axis reduction the
vector engines cannot do) which are exactly the missing lower-triangle row
sums of later tiles.  Host adds row+col parts and takes the log.

Other pieces (measured-on-HW design):
  * single input: raw x transposed in fp8e4m3 (dtype cast only on host;
    IEEE e4m3 -- max normal 240; +-448 encodes inf and NaN-poisons PSUM).
  * Gram via fp8 DoubleRow matmuls (~1.6x bf16 net); operand
    xn8 = 64*x/||x|| built by 4 chunked DVE tensor_mul (fp8*f16->fp8).
  * ssq via PE: 16 DR matmuls/batch form the raw Gram's diagonal 128x128
    blocks; the diagonal dominates those rows, so one 3D reduce_max
    extracts it.  rsqrt on GpSimd (constant-seed Newton, 2 iters).
  * rbc broadcast: rinvT [8,128] -> DRAM [1024] -> stride-0 broadcast DMA
    into [128,1024] f16 (frees PE one-hot matmuls; batch 0 keeps them to
    avoid DMA latency in the head).
  * diag self-sim masked by a DoubleRow matmul with stacked constants
    ([-240 I;0]^T [240 I;0] = -57600 I) -- no PE mode switch mid-strip.
  * PSUM: 2 strip buffers + diag-block buffer + [4,1024] column-sum
    accumulator = 8 banks exactly.
"""

import sys

import numpy as np

_TRN = "/opt/trn_rl_repo"
if _TRN not in sys.path:
    sys.path.insert(0, _TRN)

B, N, D = 32, 1024, 512
NCORES = 8
BLOC = B // NCORES  # batches per core
P = 128
NT = N // P  # row tiles (strips) per batch
KC = D // P  # contraction chunks
KP = KC // 2  # DoubleRow k-tile pairs

EPS = 1e-8
S_EXP = 256.0  # LSE sharpness (in cosine units)
C_EXP = 0.2  # LSE center: exp(s*(cos - c)) keeps f32 exp in range
G_SCALE = 4096.0  # Gram scale: both operands are 64*xn

_CACHE = {}


def build_nc():
    import concourse.bacc as bacc
    import concourse.mybir as mybir
    from concourse import masks, tile

    f32 = mybir.dt.float32
    f16 = mybir.dt.float16
    bf16 = mybir.dt.bfloat16
    fp8 = mybir.dt.float8e4
    AF = mybir.ActivationFunctionType
    ALU = mybir.AluOpType
    DR = mybir.MatmulPerfMode.DoubleRow

    nc = bacc.Bacc(
        "TRN2", target_bir_lowering=False, debug=False, num_devices=NCORES
    )
    xl_dram = nc.dram_tensor("xl", [BLOC, D, N], fp8, kind="ExternalInput")
    rb_dram = nc.dram_tensor("rb", [BLOC, N], f16, kind="Internal")
    ac_dram = nc.dram_tensor("ac", [P, BLOC * NT], f32, kind="ExternalOutput")
    cc_dram = nc.dram_tensor("cc", [BLOC, N - P], f32, kind="ExternalOutput")

    with tile.TileContext(nc) as tc:
        with (
            tc.tile_pool(name="const", bufs=1) as cpool,
            tc.tile_pool(name="xl", bufs=3) as xlpool,
            tc.tile_pool(name="xn", bufs=2) as xnpool,
            tc.tile_pool(name="rbc", bufs=2) as rbcpool,
            tc.tile_pool(name="stat", bufs=2) as spool,
            tc.tile_pool(name="escr", bufs=16) as epool,
            tc.tile_pool(name="outp", bufs=1) as opool,
            tc.tile_pool(name="gpsum", bufs=2, space="PSUM") as gpool,
            tc.tile_pool(name="dpsum", bufs=1, space="PSUM") as dpool,
            tc.tile_pool(name="cpsum", bufs=1, space="PSUM") as ccpool,
        ):
            identH = cpool.tile([P, P], f16)
            masks.make_identity(nc, identH[:])
            # DoubleRow diag-mask constants: ktile0 = +-240*I, ktile1 = 0
            negI = cpool.tile([P, 2, P], fp8)
            nc.gpsimd.memset(negI[:], 0.0)
            nc.gpsimd.affine_select(
                out=negI[:, 0], in_=negI[:, 0], compare_op=ALU.not_equal,
                fill=-240.0, base=0, pattern=[[-1, P]], channel_multiplier=1,
            )
            posI = cpool.tile([P, 2, P], fp8)
            nc.gpsimd.memset(posI[:], 0.0)
            nc.gpsimd.affine_select(
                out=posI[:, 0], in_=posI[:, 0], compare_op=ALU.not_equal,
                fill=240.0, base=0, pattern=[[-1, P]], channel_multiplier=1,
            )
            # oneh[k, t, q] = 1.0 iff k == t (head-batch rbc broadcast)
            oneh = cpool.tile([NT, NT, P], f16)
            nc.gpsimd.memset(oneh[:], 0.0)
            nc.gpsimd.affine_select(
                out=oneh[:], in_=oneh[:], compare_op=ALU.not_equal,
                fill=1.0, base=0, pattern=[[-1, NT], [0, P]],
                channel_multiplier=1,
            )
            ones = cpool.tile([P, 32], bf16)
            nc.gpsimd.memset(ones[:], 1.0)
            warm_rhs = cpool.tile([P, 512], f16)
            nc.gpsimd.memset(warm_rhs[:], 0.0)
            ebias = cpool.tile([P, 1], f32)
            nc.gpsimd.memset(ebias[:], -S_EXP * C_EXP)
            yseed = cpool.tile([P, NT], f32)
            nc.gpsimd.memset(yseed[:], 0.0442)

            ac = opool.tile([P, BLOC * NT], f32)
            cacc = ccpool.tile([96, N], f32)
            cc0 = opool.tile([32, N], f32)
            ccsb = opool.tile([96, N], f32)

            xl_r = xl_dram.ap().rearrange("b (k p) n -> b p k n", p=P)

            def warm(n):
                warm_ps = gpool.tile([P, N], f32, tag="G")
                for _ in range(n):
                    nc.tensor.matmul(warm_ps[:, :512], identH[:], warm_rhs[:])

            # Pin the exp_and_others ACT table set (covers Exp + Copy).
            pin = cpool.tile([P, 1], f32)
            nc.gpsimd.memset(pin[:], 0.0)
            nc.scalar.activation(pin[:], pin[:], AF.Exp)

            states = {b: {} for b in range(BLOC)}

            def load_xl(b, st):
                xl_all = xlpool.tile([P, KC, N], fp8, tag="xl")
                nc.sync.dma_start(xl_all[:], xl_r[b])
                st["xl"] = xl_all

            def ssq_mm(b, st):
                # raw-Gram diagonal blocks; block (t,t) diag = ssq of tile t
                dps = dpool.tile([P, N], f32, tag="D")
                xl = st["xl"]
                for t in range(NT):
                    sl = slice(t * P, (t + 1) * P)
                    for q in range(KP):
                        nc.tensor.matmul(
                            dps[:, sl],
                            xl[:, 2 * q : 2 * q + 2, sl],
                            xl[:, 2 * q : 2 * q + 2, sl],
                            start=(q == 0), stop=(q == KP - 1),
                            perf_mode=DR,
                        )
                st["dps"] = dps

            def ssq_extract(b, st):
                ssq = spool.tile([P, NT], f32, tag="ssq")
                dv = st["dps"][:].rearrange("p (t c) -> p t c", c=P)
                nc.vector.reduce_max(ssq[:], dv, axis=mybir.AxisListType.X)
                st["ssq"] = ssq

            def rsqrt(b, st):
                # y = rsqrt(ssq), Newton from a constant seed (2 iters).
                # All on the DVE: tiny [128,8] ops; a GpSimd chain here gets
                # crushed by the DVE's SBUF port lockout and delays the
                # whole rbc pipeline.
                ssq = st["ssq"]
                ya = spool.tile([P, NT], f32, tag="ya")
                yb = spool.tile([P, NT], f32, tag="yb")
                u = spool.tile([P, NT], f32, tag="u")
                w = spool.tile([P, NT], f32, tag="w")
                cur, nxt = yseed, ya
                for _ in range(2):
                    nc.vector.tensor_mul(u[:], cur[:], cur[:])
                    nc.vector.tensor_mul(u[:], u[:], ssq[:])
                    nc.vector.tensor_scalar(
                        out=w[:], in0=u[:], scalar1=-0.5, scalar2=1.5,
                        op0=ALU.mult, op1=ALU.add,
                    )
                    nc.vector.tensor_mul(nxt[:], cur[:], w[:])
                    cur, nxt = (ya, yb) if cur is yseed else (yb, ya)
                rinv16 = spool.tile([P, NT], f16, tag="rinv16")
                nc.vector.tensor_scalar_mul(rinv16[:], cur[:], 64.0)
                st["rinv16"] = rinv16

            def transpose_rinv(b, st):
                # rinvT[t, q] = rinv16[q, t], staged through a corner of the
                # (idle between uses) diag-block PSUM tile
                tr_ps = dpool.tile([P, N], f32, tag="D")
                rinvT_ps = tr_ps[:NT, : P // 2].bitcast(f16)
                nc.tensor.matmul(
                    rinvT_ps, st["rinv16"][:], identH[:], is_transpose=True
                )
                rinvT = spool.tile([NT, P], f16, tag="rinvT")
                nc.scalar.copy(rinvT[:], rinvT_ps)
                st["rinvT"] = rinvT

            def rbc_dma(b, st):
                # rinvT [8,128] -> DRAM [1024] -> broadcast to [128,1024]
                nc.sync.dma_start(
                    rb_dram.ap()[b].rearrange("(t q) -> t q", t=NT),
                    st["rinvT"][:],
                )
                rbc = rbcpool.tile([P, N], f16, tag="rbc_sb")
                nc.sync.dma_start(
                    rbc[:], rb_dram.ap()[b].unsqueeze(0).broadcast_to((P, N))
                )
                st["rbc"] = rbc

            def rbc_mm(b, st):
                # head-batch path: one-hot matmuls (PE idle there anyway)
                rbc_ps = dpool.tile([P, N], f32, tag="D")
                for t in range(NT):
                    nc.tensor.matmul(
                        rbc_ps[:, t * P : (t + 1) * P], oneh[:, t, :],
                        st["rinvT"][:],
                    )
                rbc = rbcpool.tile([P, N], f16, tag="rbc_sb")
                nc.scalar.copy(rbc[:], rbc_ps[:])
                st["rbc"] = rbc

            def scale_chunk(b, st, k):
                # xn8 = xl * rbc -> 64 * x/||x|| in e4m3 (chunk k)
                if k == 0:
                    xn8 = xnpool.tile([P, KC, N], fp8, tag="xn8")
                    st["xn8"] = xn8
                nc.vector.tensor_mul(
                    st["xn8"][:, k], st["xl"][:, k], st["rbc"][:]
                )

            def scale_half(b, st, h):
                # head path: scale column-halves (all k-chunks) so narrow
                # strips can start after the upper half alone
                if "xn8" not in st:
                    xn8 = xnpool.tile([P, KC, N], fp8, tag="xn8")
                    st["xn8"] = xn8
                cs = slice(h * 512, (h + 1) * 512)
                nc.vector.tensor_mul(
                    st["xn8"][:, :, cs], st["xl"][:, :, cs],
                    st["rbc"][:, cs].unsqueeze(1).broadcast_to((P, KC, 512)),
                )

            def strip(b, t, st):
                # G[tile-t rows, global cols t*128..1024) in PSUM cols [0, w)
                w = N - t * P
                xn8 = st["xn8"]
                G = gpool.tile([P, N], f32, tag="G")
                for q in range(KP):
                    for c0 in range(0, w, 512):
                        c1 = min(c0 + 512, w)
                        nc.tensor.matmul(
                            G[:, c0:c1],
                            xn8[:, 2 * q : 2 * q + 2, t * P : (t + 1) * P],
                            xn8[:, 2 * q : 2 * q + 2, t * P + c0 : t * P + c1],
                            start=(q == 0), stop=False,
                            perf_mode=DR,
                        )
                nc.tensor.matmul(
                    G[:, :P], negI[:], posI[:], start=False, stop=True,
                    perf_mode=DR,
                )
                esc = epool.tile([P, N], bf16, tag="esc")
                nc.scalar.activation(
                    esc[:, :w], G[:, :w], AF.Exp,
                    scale=S_EXP / G_SCALE, bias=ebias[:],
                    accum_out=ac[:, b * NT + t : b * NT + t + 1],
                )
                st.setdefault("esc", {})[t] = esc

            CBASE = {0: 0, 1: 32, 2: 64, 3: 0}

            def ones_mm(b, t, st):
                # column sums of esc (partition reduce on the PE): the
                # lower-triangle row-sum contributions for tiles > t.
                # out rows are 32 identical copies (matmul cost is per
                # column); base partition must be 0/32/64, so batch 3
                # reuses base 0 after batch 0's row is evacuated.
                w = N - t * P
                if w <= P:
                    return
                base = CBASE[b]
                for c0 in range(P, w, 512):
                    c1 = min(c0 + 512, w)
                    nc.tensor.matmul(
                        cacc[base : base + 32, t * P + c0 : t * P + c1],
                        ones[:],
                        st["esc"][t][:, c0:c1],
                        start=(t == 0), stop=(t == NT - 2),
                    )

            # ---- head ----
            load_xl(0, states[0])
            load_xl(1, states[1])
            warm(4)
            ssq_mm(0, states[0])
            ssq_extract(0, states[0])
            rsqrt(0, states[0])
            load_xl(2, states[2])
            warm(3)
            transpose_rinv(0, states[0])
            rbc_mm(0, states[0])
            scale_half(0, states[0], 1)
            scale_half(0, states[0], 0)
            ssq_mm(1, states[1])
            ssq_extract(1, states[1])
            rsqrt(1, states[1])
            transpose_rinv(1, states[1])
            rbc_dma(1, states[1])
            load_xl(3, states[3])

            # ---- steady ----
            for b in range(BLOC):
                order = range(NT - 1, -1, -1) if b == 0 else range(NT)
                # batch 0 runs its strips first (j0 shift): the head chain
                # already gated the start, so prep slots move late
                j0 = 3 if b == 0 else 0
                for i, t in enumerate(order):
                    j = i - j0
                    if j == 2 and b > 0:
                        # previous batch's column-sum matmuls: esc tiles
                        # ready, issue back-to-back on the PE
                        for tt in range(NT - 1):
                            ones_mm(b - 1, tt, states[b - 1])
                    if j == 1 and b + 2 < BLOC:
                        ssq_mm(b + 2, states[b + 2])
                    if j in (1, 2) and b + 1 < BLOC:
                        scale_chunk(b + 1, states[b + 1], j - 1)
                    elif j == 3 and b + 2 < BLOC:
                        # b+2 chain interleaved mid-scale so the rbc DMA
                        # issues mid-phase, not at the phase boundary
                        ssq_extract(b + 2, states[b + 2])
                        rsqrt(b + 2, states[b + 2])
                    elif j == 4 and b + 2 < BLOC:
                        transpose_rinv(b + 2, states[b + 2])
                        rbc_dma(b + 2, states[b + 2])
                    elif j in (5, 6) and b + 1 < BLOC:
                        scale_chunk(b + 1, states[b + 1], j - 3)
                    elif j == 6 and b == BLOC - 1:
                        nc.vector.tensor_copy(ccsb[32:64, :], cacc[32:64, :])
                        nc.vector.tensor_copy(ccsb[64:96, :], cacc[64:96, :])
                    if b == 0 and i in (6, 7):
                        scale_chunk(1, states[1], i - 4)
                    if i == 7 and b == 2:
                        nc.vector.tensor_copy(cc0[:], cacc[0:32, :])
                    strip(b, t, states[b])

            for tt in range(NT - 1):
                ones_mm(BLOC - 1, tt, states[BLOC - 1])
            nc.vector.tensor_copy(ccsb[0:32, :], cacc[0:32, :])
            nc.sync.dma_start(ac_dram.ap(), ac[:])
            # rows: b0 from cc0, b1 at 32, b2 at 64, b3 at 0 (reused)
            nc.sync.dma_start(cc_dram.ap()[0].unsqueeze(0), cc0[0:1, P:])
            nc.sync.dma_start(cc_dram.ap()[1].unsqueeze(0), ccsb[32:33, P:])
            nc.sync.dma_start(cc_dram.ap()[2].unsqueeze(0), ccsb[64:65, P:])
            nc.sync.dma_start(cc_dram.ap()[3].unsqueeze(0), ccsb[0:1, P:])

    nc.compile()
    return nc


def get_nc():
    if "nc" not in _CACHE:
        _CACHE["nc"] = build_nc()
    return _CACHE["nc"]


def shard_inputs(sparse_feats):
    import ml_dtypes

    x = np.ascontiguousarray(sparse_feats, dtype=np.float32).reshape(
        NCORES, BLOC, N, D
    )
    xt = np.ascontiguousarray(x.transpose(0, 1, 3, 2))
    xl8 = xt.astype(ml_dtypes.float8_e4m3)
    return [{"xl": xl8[c]} for c in range(NCORES)]


def finalize(ac_all, cc_all):
    """Row i = (b, t, q): LSE total = ac[q, b*8+t] (upper-triangle row sum,
    incl the masked diag block) + cc[b, t*128+q - 128] (column sums from
    earlier tiles; tile 0 has none).  m = 0.2 + ln(total)/256."""
    ac = np.asarray(ac_all, dtype=np.float64)  # [cores, 128, BLOC*NT]
    cc = np.asarray(cc_all, dtype=np.float64)  # [cores, BLOC, N-P]
    ncores = ac.shape[0]
    tot = np.empty((ncores, BLOC, NT, P))
    for b in range(BLOC):
        for t in range(NT):
            r = ac[:, :, b * NT + t]  # [cores, 128]
            if t > 0:
                r = r + cc[:, b, t * P - P : (t + 1) * P - P]
            tot[:, b, t] = r
    m = C_EXP + np.log(tot) / S_EXP
    t2 = np.maximum(2.0 - 2.0 * m, 0.0)
    dist = 0.5 * np.sqrt(t2)
    return np.float32(-np.mean(np.log(dist + EPS)))


def run_on_hw(sparse_feats, trace=False, **kw):
    from concourse.bass_utils import run_bass_kernel_spmd

    nc = get_nc()
    res = run_bass_kernel_spmd(
        nc, shard_inputs(sparse_feats), list(range(NCORES)), trace=trace, **kw
    )
    ac = np.stack([res.results[c]["ac"] for c in range(NCORES)])
    cc = np.stack([res.results[c]["cc"] for c in range(NCORES)])
    return finalize(ac, cc), res


def kernel(sparse_feats):
    loss, _ = run_on_hw(sparse_feats)
    return loss


# revision 27
# speedup vs baseline: 1.0084x; 1.0084x over previous
"""KoLeoLoss Trainium2 kernel (nn_KoLeoLoss_73538430042938) — v4.

Math: rows are L2-normalized; the loss needs, per row, the max off-diagonal
cosine sim m_i, taken here as a sharp log-sum-exp: m = 0.2 + ln(sum_j
exp(256*(cos_ij - 0.2)))/256 (bias vs the true max ~1e-4 on this data).

The LSE form makes the scan SYMMETRIC, which halves both the matmul and the
elementwise work: strip t computes G[tile t rows, cols >= t*128] only (the
upper triangle).  ACT turns each strip into esc = exp(G/16 - 51.2) (bf16)
with accum_out giving per-row partial sums; a ones-vector PE matmul over
esc[:, 128:] then yields the COLUMN sums (a partition-axis reduction the
vector engines cannot do) which are exactly the missing lower-triangle row
sums of later tiles.  Host adds row+col parts and takes the log.

Other pieces (measured-on-HW design):
  * single input: raw x transposed in fp8e4m3 (dtype cast only on host;
    IEEE e4m3 -- max normal 240; +-448 encodes inf and NaN-poisons PSUM).
  * Gram via fp8 DoubleRow matmuls (~1.6x bf16 net); operand
    xn8 = 64*x/||x|| built by 4 chunked DVE tensor_mul (fp8*f16->fp8).
  * ssq via PE: 16 DR matmuls/batch form the raw Gram's diagonal 128x128
    blocks; the diagonal dominates those rows, so one 3D reduce_max
    extracts it.  rsqrt on GpSimd (constant-seed Newton, 2 iters).
  * rbc broadcast: rinvT [8,128] -> DRAM [1024] -> stride-0 broadcast DMA
    into [128,1024] f16 (frees PE one-hot matmuls; batch 0 keeps them to
    avoid DMA latency in the head).
  * diag self-sim masked by a DoubleRow matmul with stacked constants
    ([-240 I;0]^T [240 I;0] = -57600 I) -- no PE mode switch mid-strip.
  * PSUM: 2 strip buffers + diag-block buffer + [4,1024] column-sum
    accumulator = 8 banks exactly.
"""

import sys

import numpy as np

_TRN = "/opt/trn_rl_repo"
if _TRN not in sys.path:
    sys.path.insert(0, _TRN)

B, N, D = 32, 1024, 512
NCORES = 8
BLOC = B // NCORES  # batches per core
P = 128
NT = N // P  # row tiles (strips) per batch
KC = D // P  # contraction chunks
KP = KC // 2  # DoubleRow k-tile pairs

EPS = 1e-8
S_EXP = 256.0  # LSE sharpness (in cosine units)
C_EXP = 0.2  # LSE center: exp(s*(cos - c)) keeps f32 exp in range
G_SCALE = 4096.0  # Gram scale: both operands are 64*xn

_CACHE = {}


def build_nc():
    import concourse.bacc as bacc
    import concourse.mybir as mybir
    from concourse import masks, tile

    f32 = mybir.dt.float32
    f16 = mybir.dt.float16
    bf16 = mybir.dt.bfloat16
    fp8 = mybir.dt.float8e4
    AF = mybir.ActivationFunctionType
    ALU = mybir.AluOpType
    DR = mybir.MatmulPerfMode.DoubleRow

    nc = bacc.Bacc(
        "TRN2", target_bir_lowering=False, debug=False, num_devices=NCORES
    )
    xl_dram = nc.dram_tensor("xl", [BLOC, D, N], fp8, kind="ExternalInput")
    rb_dram = nc.dram_tensor("rb", [BLOC, N], f16, kind="Internal")
    ac_dram = nc.dram_tensor("ac", [P, BLOC * NT], f32, kind="ExternalOutput")
    cc_dram = nc.dram_tensor("cc", [BLOC, N - P], f32, kind="ExternalOutput")

    with tile.TileContext(nc) as tc:
        with (
            tc.tile_pool(name="const", bufs=1) as cpool,
            tc.tile_pool(name="xl", bufs=3) as xlpool,
            tc.tile_pool(name="xn", bufs=2) as xnpool,
            tc.tile_pool(name="rbc", bufs=2) as rbcpool,
            tc.tile_pool(name="stat", bufs=2) as spool,
            tc.tile_pool(name="escr", bufs=16) as epool,
            tc.tile_pool(name="outp", bufs=1) as opool,
            tc.tile_pool(name="gpsum", bufs=2, space="PSUM") as gpool,
            tc.tile_pool(name="dpsum", bufs=1, space="PSUM") as dpool,
            tc.tile_pool(name="cpsum", bufs=1, space="PSUM") as ccpool,
        ):
            identH = cpool.tile([P, P], f16)
            masks.make_identity(nc, identH[:])
            # DoubleRow diag-mask constants: ktile0 = +-240*I, ktile1 = 0
            negI = cpool.tile([P, 2, P], fp8)
            nc.gpsimd.memset(negI[:], 0.0)
            nc.gpsimd.affine_select(
                out=negI[:, 0], in_=negI[:, 0], compare_op=ALU.not_equal,
                fill=-240.0, base=0, pattern=[[-1, P]], channel_multiplier=1,
            )
            posI = cpool.tile([P, 2, P], fp8)
            nc.gpsimd.memset(posI[:], 0.0)
            nc.gpsimd.affine_select(
                out=posI[:, 0], in_=posI[:, 0], compare_op=ALU.not_equal,
                fill=240.0, base=0, pattern=[[-1, P]], channel_multiplier=1,
            )
            # oneh[k, t, q] = 1.0 iff k == t (head-batch rbc broadcast)
            oneh = cpool.tile([NT, NT, P], f16)
            nc.gpsimd.memset(oneh[:], 0.0)
            nc.gpsimd.affine_select(
                out=oneh[:], in_=oneh[:], compare_op=ALU.not_equal,
                fill=1.0, base=0, pattern=[[-1, NT], [0, P]],
                channel_multiplier=1,
            )
            ones = cpool.tile([P, 32], bf16)
            nc.gpsimd.memset(ones[:], 1.0)
            warm_rhs = cpool.tile([P, 512], f16)
            nc.gpsimd.memset(warm_rhs[:], 0.0)
            ebias = cpool.tile([P, 1], f32)
            nc.gpsimd.memset(ebias[:], -S_EXP * C_EXP)
            yseed = cpool.tile([P, NT], f32)
            nc.gpsimd.memset(yseed[:], 0.0442)

            ac = opool.tile([P, BLOC * NT], f32)
            cacc = ccpool.tile([96, N], f32)
            cc0 = opool.tile([32, N], f32)
            ccsb = opool.tile([96, N], f32)

            xl_r = xl_dram.ap().rearrange("b (k p) n -> b p k n", p=P)

            def warm(n):
                warm_ps = gpool.tile([P, N], f32, tag="G")
                for _ in range(n):
                    nc.tensor.matmul(warm_ps[:, :512], identH[:], warm_rhs[:])

            # Pin the exp_and_others ACT table set (covers Exp + Copy).
            pin = cpool.tile([P, 1], f32)
            nc.gpsimd.memset(pin[:], 0.0)
            nc.scalar.activation(pin[:], pin[:], AF.Exp)

            states = {b: {} for b in range(BLOC)}

            def load_xl(b, st):
                xl_all = xlpool.tile([P, KC, N], fp8, tag="xl")
                nc.sync.dma_start(xl_all[:], xl_r[b])
                st["xl"] = xl_all

            def ssq_mm(b, st):
                # raw-Gram diagonal blocks; block (t,t) diag = ssq of tile t
                dps = dpool.tile([P, N], f32, tag="D")
                xl = st["xl"]
                for t in range(NT):
                    sl = slice(t * P, (t + 1) * P)
                    for q in range(KP):
                        nc.tensor.matmul(
                            dps[:, sl],
                            xl[:, 2 * q : 2 * q + 2, sl],
                            xl[:, 2 * q : 2 * q + 2, sl],
                            start=(q == 0), stop=(q == KP - 1),
                            perf_mode=DR,
                        )
                st["dps"] = dps

            def ssq_extract(b, st):
                ssq = spool.tile([P, NT], f32, tag="ssq")
                dv = st["dps"][:].rearrange("p (t c) -> p t c", c=P)
                nc.vector.reduce_max(ssq[:], dv, axis=mybir.AxisListType.X)
                st["ssq"] = ssq

            def rsqrt(b, st):
                # y = rsqrt(ssq), Newton from a constant seed (2 iters).
                # All on the DVE: tiny [128,8] ops; a GpSimd chain here gets
                # crushed by the DVE's SBUF port lockout and delays the
                # whole rbc pipeline.
                ssq = st["ssq"]
                ya = spool.tile([P, NT], f32, tag="ya")
                yb = spool.tile([P, NT], f32, tag="yb")
                u = spool.tile([P, NT], f32, tag="u")
                w = spool.tile([P, NT], f32, tag="w")
                cur, nxt = yseed, ya
                for _ in range(2):
                    nc.vector.tensor_mul(u[:], cur[:], cur[:])
                    nc.vector.tensor_mul(u[:], u[:], ssq[:])
                    nc.vector.tensor_scalar(
                        out=w[:], in0=u[:], scalar1=-0.5, scalar2=1.5,
                        op0=ALU.mult, op1=ALU.add,
                    )
                    nc.vector.tensor_mul(nxt[:], cur[:], w[:])
                    cur, nxt = (ya, yb) if cur is yseed else (yb, ya)
                rinv16 = spool.tile([P, NT], f16, tag="rinv16")
                nc.vector.tensor_scalar_mul(rinv16[:], cur[:], 64.0)
                st["rinv16"] = rinv16

            def transpose_rinv(b, st):
                # rinvT[t, q] = rinv16[q, t], staged through a corner of the
                # (idle between uses) diag-block PSUM tile
                tr_ps = dpool.tile([P, N], f32, tag="D")
                rinvT_ps = tr_ps[:NT, : P // 2].bitcast(f16)
                nc.tensor.matmul(
                    rinvT_ps, st["rinv16"][:], identH[:], is_transpose=True
                )
                rinvT = spool.tile([NT, P], f16, tag="rinvT")
                nc.scalar.copy(rinvT[:], rinvT_ps)
                st["rinvT"] = rinvT

            def rbc_dma(b, st):
                # rinvT [8,128] -> DRAM [1024] -> broadcast to [128,1024]
                nc.sync.dma_start(
                    rb_dram.ap()[b].rearrange("(t q) -> t q", t=NT),
                    st["rinvT"][:],
                )
                rbc = rbcpool.tile([P, N], f16, tag="rbc_sb")
                nc.sync.dma_start(
                    rbc[:], rb_dram.ap()[b].unsqueeze(0).broadcast_to((P, N))
                )
                st["rbc"] = rbc

            def rbc_mm(b, st):
                # head-batch path: one-hot matmuls (PE idle there anyway)
                rbc_ps = dpool.tile([P, N], f32, tag="D")
                for t in range(NT):
                    nc.tensor.matmul(
                        rbc_ps[:, t * P : (t + 1) * P], oneh[:, t, :],
                        st["rinvT"][:],
                    )
                rbc = rbcpool.tile([P, N], f16, tag="rbc_sb")
                nc.scalar.copy(rbc[:], rbc_ps[:])
                st["rbc"] = rbc

            def scale_chunk(b, st, k):
                # xn8 = xl * rbc -> 64 * x/||x|| in e4m3 (chunk k)
                if k == 0:
                    xn8 = xnpool.tile([P, KC, N], fp8, tag="xn8")
                    st["xn8"] = xn8
                nc.vector.tensor_mul(
                    st["xn8"][:, k], st["xl"][:, k], st["rbc"][:]
                )

            def scale_half(b, st, h):
                # head path: scale column-halves (all k-chunks) so narrow
                # strips can start after the upper half alone
                if "xn8" not in st:
                    xn8 = xnpool.tile([P, KC, N], fp8, tag="xn8")
                    st["xn8"] = xn8
                cs = slice(h * 512, (h + 1) * 512)
                nc.vector.tensor_mul(
                    st["xn8"][:, :, cs], st["xl"][:, :, cs],
                    st["rbc"][:, cs].unsqueeze(1).broadcast_to((P, KC, 512)),
                )

            def strip(b, t, st):
                # G[tile-t rows, global cols t*128..1024) in PSUM cols [0, w)
                w = N - t * P
                xn8 = st["xn8"]
                G = gpool.tile([P, N], f32, tag="G")
                for q in range(KP):
                    for c0 in range(0, w, 512):
                        c1 = min(c0 + 512, w)
                        nc.tensor.matmul(
                            G[:, c0:c1],
                            xn8[:, 2 * q : 2 * q + 2, t * P : (t + 1) * P],
                            xn8[:, 2 * q : 2 * q + 2, t * P + c0 : t * P + c1],
                            start=(q == 0), stop=False,
                            perf_mode=DR,
                        )
                nc.tensor.matmul(
                    G[:, :P], negI[:], posI[:], start=False, stop=True,
                    perf_mode=DR,
                )
                esc = epool.tile([P, N], bf16, tag="esc")
                nc.scalar.activation(
                    esc[:, :w], G[:, :w], AF.Exp,
                    scale=S_EXP / G_SCALE, bias=ebias[:],
                    accum_out=ac[:, b * NT + t : b * NT + t + 1],
                )
                st.setdefault("esc", {})[t] = esc

            CBASE = {0: 0, 1: 32, 2: 64, 3: 0}

            def ones_mm(b, t, st):
                # column sums of esc (partition reduce on the PE): the
                # lower-triangle row-sum contributions for tiles > t.
                # out rows are 32 identical copies (matmul cost is per
                # column); base partition must be 0/32/64, so batch 3
                # reuses base 0 after batch 0's row is evacuated.
                w = N - t * P
                if w <= P:
                    return
                base = CBASE[b]
                for c0 in range(P, w, 512):
                    c1 = min(c0 + 512, w)
                    nc.tensor.matmul(
                        cacc[base : base + 32, t * P + c0 : t * P + c1],
                        ones[:],
                        st["esc"][t][:, c0:c1],
                        start=(t == 0), stop=(t == NT - 2),
                    )

            # ---- head ----
            load_xl(0, states[0])
            load_xl(1, states[1])
            warm(4)
            ssq_mm(0, states[0])
            ssq_extract(0, states[0])
            rsqrt(0, states[0])
            load_xl(2, states[2])
            warm(3)
            transpose_rinv(0, states[0])
            rbc_mm(0, states[0])
            scale_half(0, states[0], 1)
            scale_half(0, states[0], 0)
            ssq_mm(1, states[1])
            ssq_extract(1, states[1])
            rsqrt(1, states[1])
            transpose_rinv(1, states[1])
            rbc_dma(1, states[1])
            load_xl(3, states[3])

            # ---- steady ----
            for b in range(BLOC):
                order = range(NT - 1, -1, -1) if b == 0 else range(NT)
                for i, t in enumerate(order):
                    j = i
                    if j == 2 and b > 0:
                        # previous batch's column-sum matmuls: esc tiles
                        # ready, issue back-to-back on the PE
                        for tt in range(NT - 1):
                            ones_mm(b - 1, tt, states[b - 1])
                    if j == 1 and b + 2 < BLOC:
                        ssq_mm(b + 2, states[b + 2])
                    if j in (1, 2) and b + 1 < BLOC:
                        scale_chunk(b + 1, states[b + 1], j - 1)
                    elif j == 3 and b + 2 < BLOC:
                        # b+2 chain interleaved mid-scale so the rbc DMA
                        # issues mid-phase, not at the phase boundary
                        ssq_extract(b + 2, states[b + 2])
                        rsqrt(b + 2, states[b + 2])
                    elif j == 4 and b + 2 < BLOC:
                        transpose_rinv(b + 2, states[b + 2])
                        rbc_dma(b + 2, states[b + 2])
                    elif j in (5, 6) and b + 1 < BLOC:
                        scale_chunk(b + 1, states[b + 1], j - 3)
                    elif j == 6 and b == BLOC - 1:
                        nc.vector.tensor_copy(ccsb[32:64, :], cacc[32:64, :])
                        nc.vector.tensor_copy(ccsb[64:96, :], cacc[64:96, :])
                    if i == 7 and b == 2:
                        nc.vector.tensor_copy(cc0[:], cacc[0:32, :])
                    strip(b, t, states[b])

            for tt in range(NT - 1):
                ones_mm(BLOC - 1, tt, states[BLOC - 1])
            nc.vector.tensor_copy(ccsb[0:32, :], cacc[0:32, :])
            nc.sync.dma_start(ac_dram.ap(), ac[:])
            # rows: b0 from cc0, b1 at 32, b2 at 64, b3 at 0 (reused)
            nc.sync.dma_start(cc_dram.ap()[0].unsqueeze(0), cc0[0:1, P:])
            nc.sync.dma_start(cc_dram.ap()[1].unsqueeze(0), ccsb[32:33, P:])
            nc.sync.dma_start(cc_dram.ap()[2].unsqueeze(0), ccsb[64:65, P:])
            nc.sync.dma_start(cc_dram.ap()[3].unsqueeze(0), ccsb[0:1, P:])

    nc.compile()
    return nc


def get_nc():
    if "nc" not in _CACHE:
        _CACHE["nc"] = build_nc()
    return _CACHE["nc"]


def shard_inputs(sparse_feats):
    import ml_dtypes

    x = np.ascontiguousarray(sparse_feats, dtype=np.float32).reshape(
        NCORES, BLOC, N, D
    )
    xt = np.ascontiguousarray(x.transpose(0, 1, 3, 2))
    xl8 = xt.astype(ml_dtypes.float8_e4m3)
    return [{"xl": xl8[c]} for c in range(NCORES)]


def finalize(ac_all, cc_all):
    """Row i = (b, t, q): LSE total = ac[q, b*8+t] (upper-triangle row sum,
    incl the masked diag block) + cc[b, t*128+q - 128] (column sums from
    earlier tiles; tile 0 has none).  m = 0.2 + ln(total)/256."""
    ac = np.asarray(ac_all, dtype=np.float64)  # [cores, 128, BLOC*NT]
    cc = np.asarray(cc_all, dtype=np.float64)  # [cores, BLOC, N-P]
    ncores = ac.shape[0]
    tot = np.empty((ncores, BLOC, NT, P))
    for b in range(BLOC):
        for t in range(NT):
            r = ac[:, :, b * NT + t]  # [cores, 128]
            if t > 0:
                r = r + cc[:, b, t * P - P : (t + 1) * P - P]
            tot[:, b, t] = r
    m = C_EXP + np.log(tot) / S_EXP
    t2 = np.maximum(2.0 - 2.0 * m, 0.0)
    dist = 0.5 * np.sqrt(t2)
    return np.float32(-np.mean(np.log(dist + EPS)))


def run_on_hw(sparse_feats, trace=False, **kw):
    from concourse.bass_utils import run_bass_kernel_spmd

    nc = get_nc()
    res = run_bass_kernel_spmd(
        nc, shard_inputs(sparse_feats), list(range(NCORES)), trace=trace, **kw
    )
    ac = np.stack([res.results[c]["ac"] for c in range(NCORES)])
    cc = np.stack([res.results[c]["cc"] for c in range(NCORES)])
    return finalize(ac, cc), res


def kernel(sparse_feats):
    loss, _ = run_on_hw(sparse_feats)
    return loss
